# revision 1
# baseline (speedup 1.0000x reference)
"""AttentionBlock (GroupNorm + 1x1 QKV + MHA + proj + residual) on 8 trn2 cores.

Sharding: core c -> (batch b = c//2, t-half = c%2). Each core computes all 4
heads for its 2048 query positions; k/v are computed over the full T=4096 from
the core's batch. No cross-core communication needed.

The host ROTATES each core's [C, T] input so the core's query half is always
columns 0..2047; k/v see the rotated full T (softmax over s is order
invariant as long as k_sb and vt share the order).

Layout trick: attention scores are computed TRANSPOSED, S_T[s, t] (s on
partitions), so the AV matmul needs no transposes: a[ch, t] = vT[s, ch].T @
P_T[s, t]. The softmax denominator comes from a ones-column appended to vT.
GroupNorm is folded into the QKV weights (per-channel scale/shift).

PSUM (8 banks): qk groups-of-2 [128,2,512] x bufs2 = 4 banks; AV accumulator
[128,2,512] bufs1 = 2 banks; production/proj pool [128,2,512] bufs1 = 2 banks.
k/vt production is woven into unit 0's group boundaries instead of running as
a serial upfront block; AV emission runs LAG groups behind QK so unit
boundaries (normalize/proj on the dedicated acc banks) never stall the PE.
"""

import math
from collections import deque

import numpy as np
import ml_dtypes

import concourse.bass as bass
import concourse.tile as tile
from concourse import bacc, mybir
from concourse import bass_utils

F32 = mybir.dt.float32
BF16 = mybir.dt.bfloat16
F16 = mybir.dt.float16
FP8 = mybir.dt.float8e4
I16 = mybir.dt.int16

B, C, HH, WW = 4, 256, 64, 64
T = HH * WW            # 4096
NH = 4                 # heads per batch
CH = C // NH           # 64 channels per head
G = 32                 # groupnorm groups
CPG = C // G           # 8 channels per group
EPS = 1e-5
NCORES = 8
THALF = T // 2         # 2048  t-columns per core
SCALE = 1.0 / math.sqrt(math.sqrt(CH))

P_DT = F16             # dtype of exp'd attention weights + vT
EXP_A = 1024.0 / math.log(2.0)          # Schraudolph exp2 multiplier (f16)
EXP_B = 1024.0 * (15.0 - 0.0430)        # Schraudolph exp2 offset (f16)
NSC = T // 128         # 32 s-chunks; group g == s-chunk g (both heads)
LAG = 7                # AV emission runs this many groups behind QK

_CACHE = {}
LAST_RESULTS = None


def _build_program():
    nc = bacc.Bacc("TRN2", target_bir_lowering=False, debug=False)

    d_xb = nc.dram_tensor("xb", [C, T], F16, kind="ExternalInput").ap()
    d_xqf = nc.dram_tensor("xqf", [C, THALF], F32, kind="ExternalInput").ap()
    d_wT = nc.dram_tensor("wT", [C, 3 * C], F32, kind="ExternalInput").ap()
    d_qkvb = nc.dram_tensor("qkvb", [3 * C, 1], F32, kind="ExternalInput").ap()
    d_vbrow = nc.dram_tensor("vbrow", [1, C], F32, kind="ExternalInput").ap()
    d_pjT = nc.dram_tensor("pjT", [C, C], F16, kind="ExternalInput").ap()
    d_pjb = nc.dram_tensor("pjb", [C, 1], F32, kind="ExternalInput").ap()
    d_gnw = nc.dram_tensor("gnw", [C, 1], F32, kind="ExternalInput").ap()
    d_gnb = nc.dram_tensor("gnb", [C, 1], F32, kind="ExternalInput").ap()
    d_gsel = nc.dram_tensor("gsel", [128, 2 * G], F32, kind="ExternalInput").ap()
    d_bsel = nc.dram_tensor("bsel", [G, C], F32, kind="ExternalInput").ap()
    d_out = nc.dram_tensor("out", [C, THALF], F32, kind="ExternalOutput").ap()

    with tile.TileContext(nc) as tc:
        with tc.tile_pool(name="data", bufs=1) as data, \
             tc.tile_pool(name="small", bufs=1) as small, \
             tc.tile_pool(name="work", bufs=3) as work:

            # ---- persistent SBUF tensors ----
            x = [data.tile([128, T], F16, tag=f"x{i}", name=f"x{i}") for i in range(2)]
            xqf = [data.tile([128, THALF], F32, tag=f"xqf{i}", name=f"xqf{i}") for i in range(2)]
            wt = [data.tile([128, 3 * C], F32, tag=f"wt{i}", name=f"wt{i}") for i in range(2)]
            wtf = [data.tile([128, 3 * C], F16, tag=f"wtf{i}", name=f"wtf{i}") for i in range(2)]
            pjt = [data.tile([128, C], F16, tag=f"pjt{i}", name=f"pjt{i}") for i in range(2)]
            q_sb = [data.tile([128, THALF], F16, tag=f"q{i}", name=f"q{i}") for i in range(2)]
            k_sb = [data.tile([128, T], F16, tag=f"k{i}", name=f"k{i}") for i in range(2)]
            # vT: per s-chunk, per head: 64 v-columns + 1 ones-column (+63 pad)
            vt = data.tile([128, NSC, NH, 2 * CH], P_DT, tag="vt", name="vt")
            a_sb = [data.tile([128, THALF], F16, tag=f"a{i}", name=f"a{i}") for i in range(2)]
            vbias_bc = data.tile([128, C], F32, tag="vbias_bc", name="vbias_bc")

            gnw = [small.tile([128, 1], F32, tag=f"gnw{i}", name=f"gnw{i}") for i in range(2)]
            gnb = [small.tile([128, 1], F32, tag=f"gnb{i}", name=f"gnb{i}") for i in range(2)]
            pjb = [small.tile([128, 1], F32, tag=f"pjb{i}", name=f"pjb{i}") for i in range(2)]
            qkb = [small.tile([128, 1], F32, tag=f"qkb{o}", name=f"qkb{o}") for o in range(4)]
            gsel = small.tile([128, 2 * G], F32, tag="gsel", name="gsel")
            bsel = small.tile([G, C], F32, tag="bsel", name="bsel")
            vbrow = small.tile([1, C], F32, tag="vbrow", name="vbrow")

            # Exp is the only table-based activation in the whole kernel
            # (groupnorm rstd is computed on DVE); warm its table once here.
            warm = small.tile([1, 1], F32, tag="warm", name="warm")
            nc.vector.memset(warm[:], 1.0)
            nc.scalar.activation(warm[:], warm[:], mybir.ActivationFunctionType.Exp)

            # ---- DMA: xb first (startup-critical), then weights, then the
            # rest. 512-col chunks so all 16 DMA engines work in parallel
            # (one descriptor lands on one ~20GB/s engine); issue alternately
            # from the Sync and Scalar HWDGE queues to double the issue rate.
            nc.sync.dma_start(gsel[:], d_gsel[:, :])
            nc.scalar.dma_start(bsel[:], d_bsel[:, :])
            for ch8 in range(8):
                t8 = slice(512 * ch8, 512 * (ch8 + 1))
                for i in range(2):
                    cs = slice(128 * i, 128 * (i + 1))
                    eng = nc.sync if (ch8 + i) % 2 == 0 else nc.scalar
                    eng.dma_start(x[i][:, t8], d_xb[cs, t8])
            for i in range(2):
                cs = slice(128 * i, 128 * (i + 1))
                (nc.sync if i == 0 else nc.scalar).dma_start(wt[i][:], d_wT[cs, :])
            for i in range(2):
                cs = slice(128 * i, 128 * (i + 1))
                nc.sync.dma_start(gnw[i][:], d_gnw[cs, :])
                nc.sync.dma_start(gnb[i][:], d_gnb[cs, :])
            for o in range(4):
                nc.sync.dma_start(qkb[o][:], d_qkvb[128 * o:128 * (o + 1), :])
            nc.sync.dma_start(vbrow[:], d_vbrow[:, :])
            for i in range(2):
                cs = slice(128 * i, 128 * (i + 1))
                nc.sync.dma_start(pjb[i][:], d_pjb[cs, :])
                nc.sync.dma_start(pjt[i][:], d_pjT[cs, :])
            for i in range(2):
                cs = slice(128 * i, 128 * (i + 1))
                nc.sync.dma_start(xqf[i][:], d_xqf[cs, :])  # residual-only, late

            with tc.tile_pool(name="psQK", bufs=2, space="PSUM") as psQK, \
                 tc.tile_pool(name="psA", bufs=1, space="PSUM") as psA, \
                 tc.tile_pool(name="psP", bufs=1, space="PSUM") as psP:

                def pp_tile():
                    return psP.tile([128, 2, 512], F32, tag="pp", name="pp")

                def qk_ps():
                    return psQK.tile([128, 2, 512], F32, tag="qk", name="qk")

                # PE p-state warmup: dummy matmuls keep the tensor engine
                # busy through the x-DMA wait so the fold/production phase
                # runs at full clock (DVFS ramps after ~3us busy).
                wux = small.tile([128, 512], F16, tag="wux", name="wux")
                nc.vector.memset(wux[:], 0.25)
                wu_ps = pp_tile()
                for r in range(48):
                    nc.tensor.matmul(wu_ps[0:64, 0, :], wux[:, 0:64], wux[:],
                                     start=True, stop=True)

                # ============ GroupNorm stats -> per-channel A, B ==========
                # Small fold matmuls rotate through the two psQK slots; their
                # readers are tiny DVE ops so rotation waits are negligible.
                A = [small.tile([128, 1], F32, tag=f"A{i}", name=f"A{i}") for i in range(2)]
                Bs = [small.tile([128, 1], F32, tag=f"B{i}", name=f"B{i}") for i in range(2)]
                pcs = [small.tile([128, 2], F32, tag=f"pcs{i}", name=f"pcs{i}") for i in range(2)]
                for i in range(2):
                    stats = work.tile([128, 8, 6], F32, tag="bnstats", name="bnstats")
                    for j in range(8):
                        nc.vector.bn_stats(stats[:, j, :], x[i][:, 512 * j:512 * (j + 1)])
                    mv = work.tile([128, 2], F32, tag="bnmv", name="bnmv")
                    nc.vector.bn_aggr(mv[:], stats[:])
                    # pcs = (mean, E[x^2]) per channel
                    nc.vector.tensor_copy(pcs[i][:, 0:1], mv[:, 0:1])
                    nc.vector.tensor_tensor(pcs[i][:, 1:2], mv[:, 0:1], mv[:, 0:1],
                                            mybir.AluOpType.mult)
                    nc.vector.tensor_tensor(pcs[i][:, 1:2], pcs[i][:, 1:2], mv[:, 1:2],
                                            mybir.AluOpType.add)

                # group stats [G, 2] = (mean_g, E[x^2]_g)
                grp_ps = qk_ps()[:G, 0, 0:2]
                nc.tensor.matmul(grp_ps, gsel[:, 0:G], pcs[0][:], start=True, stop=False)
                nc.tensor.matmul(grp_ps, gsel[:, G:2 * G], pcs[1][:], start=False, stop=True)

                grp_sb = small.tile([G, 2], F32, tag="grp_sb", name="grp_sb")
                nc.vector.tensor_copy(grp_sb[:], grp_ps)
                grp2 = small.tile([G, 2], F32, tag="grp2", name="grp2")  # (mean, rstd)
                var = small.tile([G, 1], F32, tag="var", name="var")
                nc.vector.tensor_copy(grp2[:, 0:1], grp_sb[:, 0:1])
                nc.vector.tensor_tensor(var[:], grp_sb[:, 0:1], grp_sb[:, 0:1],
                                        mybir.AluOpType.mult)
                nc.vector.tensor_tensor(var[:], grp_sb[:, 1:2], var[:],
                                        mybir.AluOpType.subtract)
                nc.vector.tensor_scalar(var[:], var[:], EPS, None,
                                        mybir.AluOpType.add)
                # rstd = rsqrt(var+eps) fully on DVE: magic-number seed + two
                # Newton iterations (no activation tables on the startup path)
                yr = small.tile([G, 1], F32, tag="yr", name="yr")
                t1 = small.tile([G, 1], mybir.dt.int32, tag="t1", name="t1")
                nc.vector.tensor_scalar(t1[:], var[:].bitcast(mybir.dt.int32),
                                        1, None,
                                        mybir.AluOpType.logical_shift_right)
                nc.vector.tensor_scalar(t1[:], t1[:], 0x5f3759df, -1,
                                        mybir.AluOpType.subtract,
                                        mybir.AluOpType.mult)
                nc.vector.tensor_copy(yr[:].bitcast(mybir.dt.int32), t1[:])
                ytmp = small.tile([G, 1], F32, tag="ytmp", name="ytmp")
                for _ in range(1):
                    nc.vector.tensor_tensor(ytmp[:], yr[:], yr[:],
                                            mybir.AluOpType.mult)
                    nc.vector.tensor_tensor(ytmp[:], ytmp[:], var[:],
                                            mybir.AluOpType.mult)
                    nc.vector.tensor_scalar(ytmp[:], ytmp[:], -0.5, 1.5,
                                            mybir.AluOpType.mult,
                                            mybir.AluOpType.add)
                    nc.vector.tensor_tensor(yr[:], yr[:], ytmp[:],
                                            mybir.AluOpType.mult)
                nc.vector.tensor_copy(grp2[:, 1:2], yr[:])

                # broadcast to channels; A = rstd*gn_w, B = gn_b - mean*A
                for i in range(2):
                    mb_ps = qk_ps()[:, 0, 0:2]
                    nc.tensor.matmul(mb_ps, bsel[:, 128 * i:128 * (i + 1)], grp2[:],
                                     start=True, stop=True)
                    nc.vector.tensor_tensor(A[i][:], mb_ps[:, 1:2], gnw[i][:],
                                            mybir.AluOpType.mult)
                    nc.vector.tensor_tensor(Bs[i][:], mb_ps[:, 0:1], A[i][:],
                                            mybir.AluOpType.mult)
                    nc.vector.tensor_tensor(Bs[i][:], gnb[i][:], Bs[i][:],
                                            mybir.AluOpType.subtract)
                    # folded weights
                    nc.vector.tensor_scalar_mul(wtf[i][:], wt[i][:], A[i][:])

                # folded q/k biases: qkvb[o] + sum_c wT[c,o]*B[c].  Only the
                # pair-0 biases (o=0, o=2) fold up front; o=1/o=3 are deferred
                # into the attention stream (their epilogues run much later).
                # block o: 0 = q pair0, 1 = q pair1, 2 = k pair0, 3 = k pair1
                def wofs_of(o):
                    return (0 if o < 2 else C) + 128 * (o % 2)

                def fold_qkb(o):
                    b_ps = qk_ps()[:, 0, 0:1]
                    wo = wofs_of(o)
                    nc.tensor.matmul(b_ps, wt[0][:, wo:wo + 128], Bs[0][:],
                                     start=True, stop=False)
                    nc.tensor.matmul(b_ps, wt[1][:, wo:wo + 128], Bs[1][:],
                                     start=False, stop=True)
                    nc.vector.tensor_tensor(qkb[o][:], qkb[o][:], b_ps,
                                            mybir.AluOpType.add)

                def fold_vb():
                    vb_ps = qk_ps()[0:1, 0, 0:C]
                    nc.tensor.matmul(vb_ps, Bs[0][:], wt[0][:, 2 * C:3 * C],
                                     start=True, stop=False)
                    nc.tensor.matmul(vb_ps, Bs[1][:], wt[1][:, 2 * C:3 * C],
                                     start=False, stop=True)
                    vb_row = small.tile([1, C], F32, tag="vb_row", name="vb_row")
                    nc.vector.tensor_tensor(vb_row[:], vb_ps, vbrow[:],
                                            mybir.AluOpType.add)
                    nc.gpsimd.partition_broadcast(vbias_bc[:], vb_row[:])

                nc.gpsimd.memset(vt[:, :, :, CH:CH + 1], 1.0)  # ones col (denominator)
                nc.gpsimd.memset(vt[:, :, :, CH + 1:2 * CH], 0.0)  # zero pad

                # ================= production helpers =====================
                def qk_pair(dst, blk, bias, o, t, ps=None):
                    # two 512-col t-tiles of q or k output block o.  Queries
                    # read x cols 0..2047 directly (host pre-rotated).
                    ts2 = slice(512 * t, 512 * (t + 2))
                    wo = wofs_of(blk)
                    if ps is None:
                        ps = pp_tile()
                    for u in range(2):
                        ts = slice(512 * (t + u), 512 * (t + u + 1))
                        nc.tensor.matmul(ps[:, u, :], wtf[0][:, wo:wo + 128],
                                         x[0][:, ts], start=True, stop=False)
                        nc.tensor.matmul(ps[:, u, :], wtf[1][:, wo:wo + 128],
                                         x[1][:, ts], start=False, stop=True)
                    nc.vector.tensor_scalar(dst[o][:, ts2].rearrange("p (u f) -> p u f", u=2),
                                            ps[:], bias[:], SCALE,
                                            mybir.AluOpType.add, mybir.AluOpType.mult)

                def qk_single(dst, blk, bias, o, t, ps=None):
                    # one 512-col t-tile of q or k output block o
                    ts = slice(512 * t, 512 * (t + 1))
                    wo = wofs_of(blk)
                    if ps is None:
                        ps = pp_tile()
                    nc.tensor.matmul(ps[:, 0, :], wtf[0][:, wo:wo + 128],
                                     x[0][:, ts], start=True, stop=False)
                    nc.tensor.matmul(ps[:, 0, :], wtf[1][:, wo:wo + 128],
                                     x[1][:, ts], start=False, stop=True)
                    nc.vector.tensor_scalar(dst[o][:, ts], ps[:, 0, :],
                                            bias[:], SCALE,
                                            mybir.AluOpType.add,
                                            mybir.AluOpType.mult)

                def vt_quad(sc, ps=None):
                    # four 128-wide s-chunks through one pp slot
                    ps = (pp_tile() if ps is None else ps).rearrange(
                        "p u (v f) -> p (u v) f", v=2)
                    for u in range(4):
                        ss = slice(128 * (sc + u), 128 * (sc + u + 1))
                        nc.tensor.matmul(ps[:, u, :], x[0][:, ss], wtf[0][:, 2 * C:3 * C],
                                         start=True, stop=False)
                        nc.tensor.matmul(ps[:, u, :], x[1][:, ss], wtf[1][:, 2 * C:3 * C],
                                         start=False, stop=True)
                    nc.vector.tensor_tensor(
                        vt[:, sc:sc + 4, :, 0:CH],
                        ps.rearrange("p u (h c) -> p u h c", h=NH),
                        bass.AP(tensor=vbias_bc[:].tensor, offset=vbias_bc[:].offset,
                                ap=[vbias_bc[:].ap[0], [0, 4], [CH, NH], [1, CH]]),
                        mybir.AluOpType.add)

                # minimal pre-attention production: q/k for pair 0's start +
                # the first vt quad, spread over three psum slots so the
                # matmuls run back-to-back instead of serializing on one pool.
                # Bias folds interleave so each epilogue's bias lands in time.
                fold_qkb(0)
                qk_pair(q_sb, 0, qkb[0], 0, 0)
                fold_qkb(2)
                qk_pair(k_sb, 2, qkb[2], 0, 0, ps=qk_ps())
                fold_vb()
                vt_quad(0, ps=qk_ps())

                # deferred production, keyed by global group index
                prod_at = {}

                def at(g, fn):
                    prod_at.setdefault(g, []).append(fn)

                # vt quads 1..7 at groups 4j-2 (deadline: AV(4j) at 4j+LAG)
                for j in range(1, 8):
                    at(4 * j - 2, lambda j=j: vt_quad(4 * j))
                # k pair t covers s-chunks 4t..4t+7; QK needs it at group 4t
                at(5, lambda: qk_pair(k_sb, 2, qkb[2], 0, 2))
                at(13, lambda: qk_pair(k_sb, 2, qkb[2], 0, 4))
                at(21, lambda: qk_pair(k_sb, 2, qkb[2], 0, 6))
                # both deferred bias folds together: two back-to-back psQK
                # allocations keep the qkp double-buffer parity unchanged
                at(30, lambda: (fold_qkb(3), fold_qkb(1)))
                at(33, lambda: qk_pair(q_sb, 0, qkb[0], 0, 2))
                at(36, lambda: qk_pair(k_sb, 3, qkb[3], 1, 0))
                at(40, lambda: qk_pair(k_sb, 3, qkb[3], 1, 2))
                at(45, lambda: qk_pair(k_sb, 3, qkb[3], 1, 4))
                at(48, lambda: qk_pair(k_sb, 3, qkb[3], 1, 6))
                at(52, lambda: qk_pair(q_sb, 1, qkb[1], 1, 0))
                at(56, lambda: qk_pair(q_sb, 1, qkb[1], 1, 2))

                # ================= attention + proj =======================
                with tc.tile_pool(name="pexp", bufs=LAG + 2) as pexp, \
                     tc.tile_pool(name="nrm", bufs=4) as nrm, \
                     tc.tile_pool(name="outp", bufs=3) as outp:

                    def emit_av(acc, p, sc, pe):
                        for j in range(2):
                            h = 2 * p + j
                            nc.tensor.matmul(acc[:, j, :], vt[:, sc, h, :],
                                             pe[:, j, :],
                                             start=(sc == 0), stop=(sc == NSC - 1))

                    def normalize(p, tb, acc):
                        # reciprocal reads the denominator row straight from
                        # PSUM; the two heads' chains interleave so the DVE
                        # and Pool stages pipeline instead of serializing.
                        tbs = slice(512 * tb, 512 * (tb + 1))
                        den = [nrm.tile([1, 512], F32, tag=f"den{hi}", name=f"den{hi}")
                               for hi in range(2)]
                        rec = [nrm.tile([1, 512], F32, tag=f"rec{hi}", name=f"rec{hi}")
                               for hi in range(2)]
                        bc = [nrm.tile([CH, 512], F32, tag=f"bc{hi}", name=f"bc{hi}")
                              for hi in range(2)]
                        for hi in range(2):
                            nc.vector.tensor_copy(den[hi][:], acc[CH:CH + 1, hi, :])
                        for hi in range(2):
                            nc.vector.reciprocal_approx_fast(rec[hi][:], den[hi][:])
                        for hi in range(2):
                            nc.gpsimd.partition_broadcast(bc[hi][:], rec[hi][:])
                        for hi in range(2):
                            hp = slice(64 * hi, 64 * hi + 64)
                            nc.vector.tensor_tensor(a_sb[p][hp, tbs],
                                                    acc[0:CH, hi, :],
                                                    bc[hi][:], mybir.AluOpType.mult)

                    def proj_tb(tb):
                        tbs = slice(512 * tb, 512 * (tb + 1))
                        pr = pp_tile()
                        for o in range(2):
                            nc.tensor.matmul(pr[:, o, :], pjt[0][:, 128 * o:128 * (o + 1)],
                                             a_sb[0][:, tbs], start=True, stop=False)
                            nc.tensor.matmul(pr[:, o, :], pjt[1][:, 128 * o:128 * (o + 1)],
                                             a_sb[1][:, tbs], start=False, stop=True)
                        for o in range(2):
                            res = outp.tile([128, 512], F32, tag="res", name="res")
                            # (proj + bias) + residual in a single DVE pass
                            nc.vector.scalar_tensor_tensor(res[:], pr[:, o, :],
                                                           pjb[o][:],
                                                           xqf[o][:, tbs],
                                                           mybir.AluOpType.add,
                                                           mybir.AluOpType.add)
                            nc.sync.dma_start(d_out[128 * o:128 * (o + 1), tbs], res[:])

                    pend = deque()   # (acc, p, sc, pe, post)
                    units = [(p, tb) for p in range(2) for tb in range(THALF // 512)]
                    gg = 0
                    for (p, tb) in units:
                        tbs = slice(512 * tb, 512 * (tb + 1))
                        acc = psA.tile([128, 2, 512], F32, tag="acc", name="acc")
                        for sc in range(NSC):
                            qkp = psQK.tile([128, 2, 512], F32, tag="qk", name="qk")
                            for j in range(2):
                                hp = slice(64 * j, 64 * j + 64)
                                nc.tensor.matmul(
                                    qkp[:, j, :],
                                    k_sb[p][hp, 128 * sc:128 * (sc + 1)],
                                    q_sb[p][hp, tbs],
                                    start=True, stop=True)
                            pe = pexp.tile([128, 2, 512], P_DT, tag="pe", name="pe")
                            if sc % 4 == 3:
                                # DVE exp2 bit-trick: bf16 bits =
                                # round(128*(x*log2e + 127 - c))
                                nc.vector.tensor_scalar(
                                    pe[:].bitcast(I16), qkp[:],
                                    EXP_A, EXP_B,
                                    mybir.AluOpType.mult, mybir.AluOpType.add)
                            else:
                                nc.scalar.activation(
                                    pe[:], qkp[:],
                                    mybir.ActivationFunctionType.Exp)
                            for fn in prod_at.pop(gg, []):
                                fn()
                            post = None
                            if sc == NSC - 1:
                                def post(p=p, tb=tb, acc=acc):
                                    normalize(p, tb, acc)
                                    if p == 1:
                                        proj_tb(tb)
                            pend.append((acc, p, sc, pe, post))
                            # taper-drain: pop 2 per group near the unit end so
                            # the lag is zero at the boundary and normalize can
                            # free the acc banks before the next unit's AVs.
                            npop = 2 if sc >= NSC - LAG else (
                                1 if len(pend) > LAG else 0)
                            for _ in range(npop):
                                if not pend:
                                    break
                                ent = pend.popleft()
                                emit_av(*ent[:4])
                                if ent[4] is not None:
                                    ent[4]()
                            gg += 1
                    while pend:
                        ent = pend.popleft()
                        emit_av(*ent[:4])
                        if ent[4] is not None:
                            ent[4]()

    nc.compile()
    return nc


def _host_consts():
    g1 = np.zeros((128, G), dtype=np.float32)
    g2 = np.zeros((128, G), dtype=np.float32)
    for c in range(128):
        g1[c, c // CPG] = 1.0 / CPG
        g2[c, G // 2 + c // CPG] = 1.0 / CPG
    gsel = np.concatenate([g1, g2], axis=1)          # [128, 2G]
    bsel = np.zeros((G, C), dtype=np.float32)
    for c in range(C):
        bsel[c // CPG, c] = 1.0
    return gsel, bsel


def kernel(x, gn_w, gn_b, qkv_w, qkv_b, proj_w, proj_b):
    global LAST_RESULTS
    if "nc" not in _CACHE:
        _CACHE["nc"] = _build_program()
    nc = _CACHE["nc"]

    x = np.ascontiguousarray(np.asarray(x, dtype=np.float32))
    xr = x.reshape(B, C, T)
    gsel, bsel = _host_consts()
    shared = {
        "wT": np.ascontiguousarray(np.asarray(qkv_w, np.float32).T),
        "qkvb": np.asarray(qkv_b, np.float32).reshape(3 * C, 1).copy(),
        "vbrow": np.asarray(qkv_b, np.float32)[2 * C:].reshape(1, C).copy(),
        "pjT": np.ascontiguousarray(np.asarray(proj_w, np.float32).T.astype(np.float16)),
        "pjb": np.asarray(proj_b, np.float32).reshape(C, 1).copy(),
        "gnw": np.asarray(gn_w, np.float32).reshape(C, 1).copy(),
        "gnb": np.asarray(gn_b, np.float32).reshape(C, 1).copy(),
        "gsel": gsel,
        "bsel": bsel,
    }
    in_maps = []
    for c in range(NCORES):
        b, hf = c // 2, c % 2
        m = dict(shared)
        # rotate so this core's query half is always columns 0..2047
        xrot = np.roll(xr[b], -hf * THALF, axis=1)
        m["xb"] = np.ascontiguousarray(xrot.astype(np.float16))
        m["xqf"] = np.ascontiguousarray(xrot[:, :THALF])
        in_maps.append(m)

    res = bass_utils.run_bass_kernel_spmd(nc, in_maps, core_ids=list(range(NCORES)))
    LAST_RESULTS = res

    out = np.empty((B, C, T), dtype=np.float32)
    for c in range(NCORES):
        b, hf = c // 2, c % 2
        out[b][:, hf * THALF:(hf + 1) * THALF] = res.results[c]["out"]
    return out.reshape(B, C, HH, WW)



# revision 5
# speedup vs baseline: 1.1622x; 1.1622x over previous
"""AttentionBlock (GroupNorm + 1x1 QKV + MHA + proj + residual) on 8 trn2 cores.

Sharding: core c -> (batch b = c//2, t-half = c%2). Each core computes all 4
heads for its 2048 query positions; k/v are computed over the full T=4096 from
the core's batch. No cross-core communication needed. The host ROTATES each
core's [C, T] input so the core's query half is always columns 0..2047.

v2 design (vs f16 baseline, 331.5us):
- AV matmuls use fp8e4 DoubleRow (contraction 256 per MM): vT and the exp'd
  attention weights (pe) are fp8e4; one DoubleRow MM replaces two f16 MMs
  (measured 216ns per MM, LDWEIGHTS 130 cols hidden).
- exp is the throughput wall (33.5M elements/core); it runs on BOTH psum-
  capable engines: ACT (table Exp -> fp8, whole [128,2,512] chunk) and DVE
  (u8-Schraudolph bit-trick: fp8e4 bits = round(11.54*x + B), saturating
  u8 convert gives exact 0 for tiny weights). GPSIMD cannot read PSUM, so it
  only does the SBUF-side reciprocal broadcast.
- softmax shift M0=2.25 keeps fp8 in range (max logit ~7.0 -> byte 111,
  NaN zone starts 120; logits < M0-4.82 round to weight 0, dropped mass
  ~2e-4 of the denominator) and cancels exactly in the normalize ratio.
- SCALE is folded into the q/k weight columns host-side; the GroupNorm v-bias
  is folded into the proj bias ON DEVICE (pjb_eff = pjb + proj_w @ vb), so vt
  production is a pure psum->fp8 copy that ACT can run.

PSUM (8 banks): ps2 [128,2,512] x3 bufs (QK/exp chunks, production pairs,
vt halves, folds, proj) + acc [128,2,512] x1 (AV accumulator; denominator
row on partition 64 via a ones-column in vT).
"""

import math
from collections import deque

import numpy as np
import ml_dtypes

import concourse.bass as bass
import concourse.tile as tile
from concourse import bacc, mybir
from concourse import bass_utils

F32 = mybir.dt.float32
F16 = mybir.dt.float16
FP8 = mybir.dt.float8e4
U8 = mybir.dt.uint8

B, C, HH, WW = 4, 256, 64, 64
T = HH * WW            # 4096
NH = 4                 # heads per batch
CH = C // NH           # 64 channels per head
G = 32                 # groupnorm groups
CPG = C // G           # 8 channels per group
EPS = 1e-5
NCORES = 8
THALF = T // 2         # 2048  t-columns per core
SCALE = 1.0 / math.sqrt(math.sqrt(CH))
NSC = T // 128         # 32 s-chunks per unit
NPAIR = NSC // 2       # 16 DoubleRow s-pairs
VSPAN = 80             # vt8 per-head column span (16B-aligned pair stride)

M0 = 2.25                                  # softmax shift (cancels in ratio)
EXP_A8 = 8.0 * math.log2(math.e)           # fp8e4m3 Schraudolph multiplier
EXP_B8 = 8.0 * (7.0 - 0.0430) - EXP_A8 * M0

LP = 5                 # AV emission lags this many s-pairs behind QK

_CACHE = {}
LAST_RESULTS = None


def _make_pattern(n_a, n_d):
    """32-slot exp-engine assignment ('A' = ACT, 'D' = DVE); starts 'AA' so
    DVE can drain the previous unit's normalize before its first chunk."""
    assert n_a + n_d == NSC
    out = ["A", "A"]
    rem = {"A": n_a - 2, "D": n_d}
    acc = {k: 0.0 for k in rem}
    tot = sum(rem.values())
    for _ in range(tot):
        counts = {k: sum(1 for c in out[2:] if c == k) for k in rem}
        for k in rem:
            acc[k] += rem[k] / tot
        avail = [k for k in rem if counts[k] < rem[k]]
        pick = max(avail, key=lambda k: acc[k])
        out.append(pick)
        acc[pick] -= 1.0
    assert len(out) == NSC
    return out


UNIT_PATTERNS = []
for u in range(8):
    if u <= 2:
        UNIT_PATTERNS.append(_make_pattern(16, 16))
    elif u == 3:
        UNIT_PATTERNS.append(_make_pattern(17, 15))
    else:
        UNIT_PATTERNS.append(_make_pattern(18, 14))


def _build_program():
    nc = bacc.Bacc("TRN2", target_bir_lowering=False, debug=False)

    d_xb = nc.dram_tensor("xb", [C, T], F16, kind="ExternalInput").ap()
    d_xqf = nc.dram_tensor("xqf", [C, THALF], F32, kind="ExternalInput").ap()
    d_wT = nc.dram_tensor("wT", [C, 3 * C], F32, kind="ExternalInput").ap()
    d_qkvb = nc.dram_tensor("qkvb", [2 * C, 1], F32, kind="ExternalInput").ap()
    d_vbrow = nc.dram_tensor("vbrow", [1, C], F32, kind="ExternalInput").ap()
    d_pjT = nc.dram_tensor("pjT", [C, C], F16, kind="ExternalInput").ap()
    d_pjb = nc.dram_tensor("pjb", [C, 1], F32, kind="ExternalInput").ap()
    d_gnw = nc.dram_tensor("gnw", [C, 1], F32, kind="ExternalInput").ap()
    d_gnb = nc.dram_tensor("gnb", [C, 1], F32, kind="ExternalInput").ap()
    d_gsel = nc.dram_tensor("gsel", [128, 2 * G], F32, kind="ExternalInput").ap()
    d_bsel = nc.dram_tensor("bsel", [G, C], F32, kind="ExternalInput").ap()
    d_scr = nc.dram_tensor("scr", [1, C], F16, kind="Internal").ap()
    d_out = nc.dram_tensor("out", [C, THALF], F32, kind="ExternalOutput").ap()
    d_dbg_q = nc.dram_tensor("dbg_q", [C, THALF], F16, kind="ExternalOutput").ap()
    d_dbg_k = nc.dram_tensor("dbg_k", [C, T], F16, kind="ExternalOutput").ap()
    d_dbg_vt = nc.dram_tensor("dbg_vt", [128, NSC * NH * VSPAN], mybir.dt.uint8, kind="ExternalOutput").ap()
    d_dbg_a = nc.dram_tensor("dbg_a", [C, THALF], F16, kind="ExternalOutput").ap()
    d_dbg_pjb = nc.dram_tensor("dbg_pjb", [C, 1], F32, kind="ExternalOutput").ap()
    d_dbg_qkb = nc.dram_tensor("dbg_qkb", [2 * C, 1], F32, kind="ExternalOutput").ap()

    with tile.TileContext(nc) as tc:
        with tc.tile_pool(name="data", bufs=1) as data, \
             tc.tile_pool(name="small", bufs=1) as small, \
             tc.tile_pool(name="work", bufs=3) as work:

            # ---- persistent SBUF tensors ----
            x = [data.tile([128, T], F16, tag=f"x{i}", name=f"x{i}") for i in range(2)]
            xqf = [data.tile([128, THALF], F32, tag=f"xqf{i}", name=f"xqf{i}") for i in range(2)]
            wt = [data.tile([128, 3 * C], F32, tag=f"wt{i}", name=f"wt{i}") for i in range(2)]
            wtf = [data.tile([128, 3 * C], F16, tag=f"wtf{i}", name=f"wtf{i}") for i in range(2)]
            pjt = [data.tile([128, C], F16, tag=f"pjt{i}", name=f"pjt{i}") for i in range(2)]
            q_sb = [data.tile([128, THALF], F16, tag=f"q{i}", name=f"q{i}") for i in range(2)]
            k_sb = [data.tile([128, T], F16, tag=f"k{i}", name=f"k{i}") for i in range(2)]
            # vT fp8: per s-chunk, per head: 64 v-columns + 1 ones-column
            vt8 = data.tile([128, NSC, NH, VSPAN], FP8, tag="vt8", name="vt8")
            a_sb = [data.tile([128, THALF], F16, tag=f"a{i}", name=f"a{i}") for i in range(2)]

            gnw = [small.tile([128, 1], F32, tag=f"gnw{i}", name=f"gnw{i}") for i in range(2)]
            gnb = [small.tile([128, 1], F32, tag=f"gnb{i}", name=f"gnb{i}") for i in range(2)]
            pjb = [small.tile([128, 1], F32, tag=f"pjb{i}", name=f"pjb{i}") for i in range(2)]
            qkb = [small.tile([128, 1], F32, tag=f"qkb{o}", name=f"qkb{o}") for o in range(4)]
            gsel = small.tile([128, 2 * G], F32, tag="gsel", name="gsel")
            bsel = small.tile([G, C], F32, tag="bsel", name="bsel")
            vbrow = small.tile([1, C], F32, tag="vbrow", name="vbrow")
            vbT = [small.tile([128, 1], F16, tag=f"vbT{i}", name=f"vbT{i}") for i in range(2)]
            nbias = small.tile([128, 1], F32, tag="nbias", name="nbias")
            nc.vector.memset(nbias[:], -M0)

            # warm the ACT Exp table once
            warm = small.tile([1, 1], F32, tag="warm", name="warm")
            nc.vector.memset(warm[:], 1.0)
            nc.scalar.activation(warm[:], warm[:], mybir.ActivationFunctionType.Exp)

            # ---- DMA: xb first (startup-critical), weights, then the rest.
            nc.sync.dma_start(gsel[:], d_gsel[:, :])
            nc.scalar.dma_start(bsel[:], d_bsel[:, :])
            for ch8 in range(8):
                t8 = slice(512 * ch8, 512 * (ch8 + 1))
                for i in range(2):
                    cs = slice(128 * i, 128 * (i + 1))
                    eng = nc.sync if (ch8 + i) % 2 == 0 else nc.scalar
                    eng.dma_start(x[i][:, t8], d_xb[cs, t8])
            for i in range(2):
                cs = slice(128 * i, 128 * (i + 1))
                (nc.sync if i == 0 else nc.scalar).dma_start(wt[i][:], d_wT[cs, :])
            for i in range(2):
                cs = slice(128 * i, 128 * (i + 1))
                nc.sync.dma_start(gnw[i][:], d_gnw[cs, :])
                nc.sync.dma_start(gnb[i][:], d_gnb[cs, :])
            for o in range(4):
                ob = slice(128 * o, 128 * (o + 1))
                nc.sync.dma_start(qkb[o][:], d_qkvb[ob, :])
            nc.sync.dma_start(vbrow[:], d_vbrow[:, :])
            for i in range(2):
                cs = slice(128 * i, 128 * (i + 1))
                nc.sync.dma_start(pjb[i][:], d_pjb[cs, :])
                nc.sync.dma_start(pjt[i][:], d_pjT[cs, :])
            for i in range(2):
                cs = slice(128 * i, 128 * (i + 1))
                nc.sync.dma_start(xqf[i][:], d_xqf[cs, :])  # residual-only, late

            with tc.tile_pool(name="ps2", bufs=3, space="PSUM") as ps2, \
                 tc.tile_pool(name="psA", bufs=1, space="PSUM") as psA:

                def ps2_tile():
                    return ps2.tile([128, 2, 512], F32, tag="p2", name="p2")

                # PE p-state warmup through the x-DMA wait
                wux = small.tile([128, 512], F16, tag="wux", name="wux")
                nc.vector.memset(wux[:], 0.25)
                wu_ps = ps2_tile()
                for r in range(48):
                    nc.tensor.matmul(wu_ps[0:64, 0, :], wux[:, 0:64], wux[:],
                                     start=True, stop=True)

                # ============ GroupNorm stats -> per-channel A, B ==========
                A = [small.tile([128, 1], F32, tag=f"A{i}", name=f"A{i}") for i in range(2)]
                Bs = [small.tile([128, 1], F32, tag=f"B{i}", name=f"B{i}") for i in range(2)]
                pcs = [small.tile([128, 2], F32, tag=f"pcs{i}", name=f"pcs{i}") for i in range(2)]
                for i in range(2):
                    stats = work.tile([128, 8, 6], F32, tag="bnstats", name="bnstats")
                    for j in range(8):
                        nc.vector.bn_stats(stats[:, j, :], x[i][:, 512 * j:512 * (j + 1)])
                    mv = work.tile([128, 2], F32, tag="bnmv", name="bnmv")
                    nc.vector.bn_aggr(mv[:], stats[:])
                    nc.vector.tensor_copy(pcs[i][:, 0:1], mv[:, 0:1])
                    nc.vector.tensor_tensor(pcs[i][:, 1:2], mv[:, 0:1], mv[:, 0:1],
                                            mybir.AluOpType.mult)
                    nc.vector.tensor_tensor(pcs[i][:, 1:2], pcs[i][:, 1:2], mv[:, 1:2],
                                            mybir.AluOpType.add)

                grp_full = ps2_tile()
                grp_ps = grp_full[0:G, 0, 0:2]
                nc.tensor.matmul(grp_ps, gsel[:, 0:G], pcs[0][:], start=True, stop=False)
                nc.tensor.matmul(grp_ps, gsel[:, G:2 * G], pcs[1][:], start=False, stop=True)

                grp_sb = small.tile([G, 2], F32, tag="grp_sb", name="grp_sb")
                nc.vector.tensor_copy(grp_sb[:], grp_ps)
                grp2 = small.tile([G, 2], F32, tag="grp2", name="grp2")
                var = small.tile([G, 1], F32, tag="var", name="var")
                nc.vector.tensor_copy(grp2[:, 0:1], grp_sb[:, 0:1])
                nc.vector.tensor_tensor(var[:], grp_sb[:, 0:1], grp_sb[:, 0:1],
                                        mybir.AluOpType.mult)
                nc.vector.tensor_tensor(var[:], grp_sb[:, 1:2], var[:],
                                        mybir.AluOpType.subtract)
                nc.vector.tensor_scalar(var[:], var[:], EPS, None,
                                        mybir.AluOpType.add)
                # rstd = rsqrt(var+eps) on DVE: magic seed + Newton iteration
                yr = small.tile([G, 1], F32, tag="yr", name="yr")
                t1 = small.tile([G, 1], mybir.dt.int32, tag="t1", name="t1")
                nc.vector.tensor_scalar(t1[:], var[:].bitcast(mybir.dt.int32),
                                        1, None,
                                        mybir.AluOpType.logical_shift_right)
                nc.vector.tensor_scalar(t1[:], t1[:], 0x5f3759df, -1,
                                        mybir.AluOpType.subtract,
                                        mybir.AluOpType.mult)
                nc.vector.tensor_copy(yr[:].bitcast(mybir.dt.int32), t1[:])
                ytmp = small.tile([G, 1], F32, tag="ytmp", name="ytmp")
                for _ in range(1):
                    nc.vector.tensor_tensor(ytmp[:], yr[:], yr[:],
                                            mybir.AluOpType.mult)
                    nc.vector.tensor_tensor(ytmp[:], ytmp[:], var[:],
                                            mybir.AluOpType.mult)
                    nc.vector.tensor_scalar(ytmp[:], ytmp[:], -0.5, 1.5,
                                            mybir.AluOpType.mult,
                                            mybir.AluOpType.add)
                    nc.vector.tensor_tensor(yr[:], yr[:], ytmp[:],
                                            mybir.AluOpType.mult)
                nc.vector.tensor_copy(grp2[:, 1:2], yr[:])

                # broadcast to channels; A = rstd*gn_w, B = gn_b - mean*A
                for i in range(2):
                    mb_full = ps2_tile()
                    mb_ps = mb_full[:, 0, 0:2]
                    nc.tensor.matmul(mb_ps, bsel[:, 128 * i:128 * (i + 1)], grp2[:],
                                     start=True, stop=True)
                    nc.vector.tensor_tensor(A[i][:], mb_ps[:, 1:2], gnw[i][:],
                                            mybir.AluOpType.mult)
                    nc.vector.tensor_tensor(Bs[i][:], mb_ps[:, 0:1], A[i][:],
                                            mybir.AluOpType.mult)
                    nc.vector.tensor_tensor(Bs[i][:], gnb[i][:], Bs[i][:],
                                            mybir.AluOpType.subtract)
                    nc.vector.tensor_scalar_mul(wtf[i][:], wt[i][:], A[i][:])

                # folded q/k biases (host pre-scaled by SCALE):
                # block o: 0 = q pair0, 1 = q pair1, 2 = k pair0, 3 = k pair1
                def wofs_of(o):
                    return (0 if o < 2 else C) + 128 * (o % 2)

                def fold_qkb(o):
                    b_full = ps2_tile()
                    b_ps = b_full[:, 0, 0:1]
                    wo = wofs_of(o)
                    nc.tensor.matmul(b_ps, wt[0][:, wo:wo + 128], Bs[0][:],
                                     start=True, stop=False)
                    nc.tensor.matmul(b_ps, wt[1][:, wo:wo + 128], Bs[1][:],
                                     start=False, stop=True)
                    nc.vector.tensor_tensor(qkb[o][:], qkb[o][:], b_ps,
                                            mybir.AluOpType.add)

                def fold_vb():
                    # vb = W_v @ B + b_v  (the full per-channel v bias), sent
                    # through DRAM to transpose [1,C] -> 2x[128,1]
                    vb_full = ps2_tile()
                    vb_ps = vb_full[0:1, 0, 0:C]
                    nc.tensor.matmul(vb_ps, Bs[0][:], wt[0][:, 2 * C:3 * C],
                                     start=True, stop=False)
                    nc.tensor.matmul(vb_ps, Bs[1][:], wt[1][:, 2 * C:3 * C],
                                     start=False, stop=True)
                    vb16 = small.tile([1, C], F16, tag="vb16", name="vb16")
                    nc.vector.scalar_tensor_tensor(vb16[:], vb_ps, 1.0, vbrow[:],
                                                   mybir.AluOpType.mult,
                                                   mybir.AluOpType.add)
                    nc.sync.dma_start(d_scr[0:1, :], vb16[:])
                    for i in range(2):
                        nc.sync.dma_start(vbT[i][:], d_scr[0:1, 128 * i:128 * (i + 1)])

                def fold_pjb():
                    # pjb_eff = pjb + proj_w @ vb
                    pe_full = ps2_tile()
                    for o in range(2):
                        pb_ps = pe_full[:, o, 0:1]
                        nc.tensor.matmul(pb_ps, pjt[0][:, 128 * o:128 * (o + 1)],
                                         vbT[0][:], start=True, stop=False)
                        nc.tensor.matmul(pb_ps, pjt[1][:, 128 * o:128 * (o + 1)],
                                         vbT[1][:], start=False, stop=True)
                    for o in range(2):
                        nc.vector.tensor_tensor(pjb[o][:], pjb[o][:],
                                                pe_full[:, o, 0:1],
                                                mybir.AluOpType.add)

                nc.gpsimd.memset(vt8[:, :, :, CH:CH + 1], 1.0)  # denominator col

                # ================= production helpers =====================
                def qk_pair(dst, blk, bias, o, t):
                    # two 512-col t-tiles of q or k output block blk -> dst[o]
                    ts2 = slice(512 * t, 512 * (t + 2))
                    wo = wofs_of(blk)
                    ps = ps2_tile()
                    for u in range(2):
                        ts = slice(512 * (t + u), 512 * (t + u + 1))
                        nc.tensor.matmul(ps[:, u, :], wtf[0][:, wo:wo + 128],
                                         x[0][:, ts], start=True, stop=False)
                        nc.tensor.matmul(ps[:, u, :], wtf[1][:, wo:wo + 128],
                                         x[1][:, ts], start=False, stop=True)
                    dv = dst[o][:, ts2].rearrange("p (u f) -> p u f", u=2)
                    nc.scalar.activation(dv, ps[:],
                                         mybir.ActivationFunctionType.Identity,
                                         bias=bias[:], scale=1.0)

                def vt_tile(v):
                    # s-pair v: vT fp8 for s-chunks 2v, 2v+1 (all 4 heads);
                    # pure copy (v bias folded into pjb_eff)
                    ps = ps2_tile()[:, 0, :].rearrange("p (u f) -> p u f", u=2)
                    for u in range(2):
                        ss = slice(128 * (2 * v + u), 128 * (2 * v + u + 1))
                        nc.tensor.matmul(ps[:, u, :], x[0][:, ss],
                                         wtf[0][:, 2 * C:3 * C],
                                         start=True, stop=False)
                        nc.tensor.matmul(ps[:, u, :], x[1][:, ss],
                                         wtf[1][:, 2 * C:3 * C],
                                         start=False, stop=True)
                    nc.scalar.activation(
                        vt8[:, 2 * v:2 * v + 2, :, 0:CH],
                        ps.rearrange("p u (h c) -> p u h c", h=NH),
                        mybir.ActivationFunctionType.Copy)

                # deferred production, keyed by global group index
                prod_at = {}

                def at(g, fn):
                    prod_at.setdefault(g, []).append(fn)

                # minimal pre-attention production
                fold_qkb(0)
                qk_pair(q_sb, 0, qkb[0], 0, 0)
                fold_qkb(2)
                qk_pair(k_sb, 2, qkb[2], 0, 0)
                fold_vb()
                vt_tile(0)

                # unit 0: vt pairs 1..15 + k0 tiles 1..3
                for v in range(1, 16):
                    at(2 * v - 2, lambda v=v: vt_tile(v))
                at(4, lambda: qk_pair(k_sb, 2, qkb[2], 0, 2))
                at(10, lambda: qk_pair(k_sb, 2, qkb[2], 0, 4))
                at(18, lambda: qk_pair(k_sb, 2, qkb[2], 0, 6))
                at(26, lambda: fold_pjb())
                # units 1-3: q0 second half, k1, q1
                at(34, lambda: qk_pair(q_sb, 0, qkb[0], 0, 2))
                at(40, lambda: (fold_qkb(3), fold_qkb(1)))
                at(44, lambda: qk_pair(k_sb, 3, qkb[3], 1, 0))
                at(52, lambda: qk_pair(k_sb, 3, qkb[3], 1, 2))
                at(66, lambda: qk_pair(k_sb, 3, qkb[3], 1, 4))
                at(74, lambda: qk_pair(k_sb, 3, qkb[3], 1, 6))
                at(80, lambda: qk_pair(q_sb, 1, qkb[1], 1, 0))
                at(100, lambda: qk_pair(q_sb, 1, qkb[1], 1, 2))

                # ================= attention + proj =======================
                with tc.tile_pool(name="pexp", bufs=LP + 3) as pexp, \
                     tc.tile_pool(name="nrm", bufs=2) as nrm, \
                     tc.tile_pool(name="outp", bufs=2) as outp:

                    def emit_av(acc, p, g, pe):
                        for j in range(2):
                            h = 2 * p + j
                            nc.tensor.matmul(
                                acc[0:CH + 1, j, :],
                                vt8[:, 2 * g:2 * g + 2, h, 0:CH + 1],
                                pe[:, :, j, :],
                                start=(g == 0), stop=(g == NPAIR - 1),
                                perf_mode=mybir.MatmulPerfMode.DoubleRow)

                    def normalize(p, tb, acc):
                        tbs = slice(512 * tb, 512 * (tb + 1))
                        # custom DVE ops don't partition-shift: copy the
                        # denominator row to partition 0 first, then recip
                        den = nrm.tile([1, 2, 512], F32, tag="den", name="den")
                        nc.vector.tensor_copy(den[:], acc[CH:CH + 1, :, :])
                        rcp = nrm.tile([1, 2, 512], F32, tag="rcp", name="rcp")
                        nc.vector.reciprocal_approx_fast(rcp[:], den[:])
                        bc = nrm.tile([CH, 2, 512], F32, tag="bc", name="bc")
                        nc.gpsimd.partition_broadcast(bc[:], rcp[:])
                        for j in range(2):
                            hp = slice(64 * j, 64 * j + 64)
                            nc.vector.tensor_tensor(a_sb[p][hp, tbs],
                                                    acc[0:CH, j, :],
                                                    bc[:, j, :],
                                                    mybir.AluOpType.mult)

                    def proj_tb(tb):
                        tbs = slice(512 * tb, 512 * (tb + 1))
                        pr = ps2_tile()
                        for o in range(2):
                            nc.tensor.matmul(pr[:, o, :], pjt[0][:, 128 * o:128 * (o + 1)],
                                             a_sb[0][:, tbs], start=True, stop=False)
                            nc.tensor.matmul(pr[:, o, :], pjt[1][:, 128 * o:128 * (o + 1)],
                                             a_sb[1][:, tbs], start=False, stop=True)
                        for o in range(2):
                            res = outp.tile([128, 512], F32, tag="res", name="res")
                            nc.vector.scalar_tensor_tensor(res[:], pr[:, o, :],
                                                           pjb[o][:],
                                                           xqf[o][:, tbs],
                                                           mybir.AluOpType.add,
                                                           mybir.AluOpType.add)
                            nc.sync.dma_start(d_out[128 * o:128 * (o + 1), tbs], res[:])

                    pend = deque()   # (acc, p, g, pe_tile, post)
                    units = [(p, tb) for p in range(2) for tb in range(THALF // 512)]
                    gg = 0
                    for ui, (p, tb) in enumerate(units):
                        pattern = UNIT_PATTERNS[ui]
                        tbs = slice(512 * tb, 512 * (tb + 1))
                        acc = psA.tile([128, 2, 512], F32, tag="acc", name="acc")
                        pe_cur = None
                        for sc in range(NSC):
                            par = sc % 2
                            g = sc // 2
                            if par == 0:
                                pe_cur = pexp.tile([128, 2, 2, 512], FP8,
                                                   tag="pe", name="pe")
                            scs = slice(128 * sc, 128 * (sc + 1))
                            qt = ps2_tile()
                            for j in range(2):
                                hp = slice(64 * j, 64 * j + 64)
                                nc.tensor.matmul(qt[:, j, :], k_sb[p][hp, scs],
                                                 q_sb[p][hp, tbs],
                                                 start=True, stop=True)
                            if pattern[sc] == "A":
                                nc.scalar.activation(
                                    pe_cur[:, par, :, :], qt[:],
                                    mybir.ActivationFunctionType.Exp,
                                    bias=nbias[:], scale=1.0)
                            else:
                                nc.vector.tensor_scalar(
                                    pe_cur[:, par, :, :].bitcast(U8), qt[:],
                                    EXP_A8, EXP_B8,
                                    mybir.AluOpType.mult, mybir.AluOpType.add)
                            for fn in prod_at.pop(gg, []):
                                fn()
                            if par == 1:
                                post = None
                                if g == NPAIR - 1:
                                    def post(p=p, tb=tb, acc=acc):
                                        normalize(p, tb, acc)
                                        if p == 1:
                                            proj_tb(tb)
                                pend.append((acc, p, g, pe_cur, post))
                                npop = 2 if g >= NPAIR - LP else (
                                    1 if len(pend) > LP else 0)
                                for _ in range(npop):
                                    if not pend:
                                        break
                                    ent = pend.popleft()
                                    emit_av(*ent[:4])
                                    if ent[4] is not None:
                                        ent[4]()
                            gg += 1
                    while pend:
                        ent = pend.popleft()
                        emit_av(*ent[:4])
                        if ent[4] is not None:
                            ent[4]()
                    for i in range(2):
                        cs = slice(128 * i, 128 * (i + 1))
                        nc.sync.dma_start(d_dbg_q[cs, :], q_sb[i][:])
                        nc.sync.dma_start(d_dbg_k[cs, :], k_sb[i][:])
                        nc.sync.dma_start(d_dbg_a[cs, :], a_sb[i][:])
                        nc.sync.dma_start(d_dbg_pjb[cs, :], pjb[i][:])
                    for o in range(4):
                        ob = slice(128 * o, 128 * (o + 1))
                        nc.sync.dma_start(d_dbg_qkb[ob, :], qkb[o][:])
                    nc.sync.dma_start(d_dbg_vt[:, :], vt8[:].bitcast(mybir.dt.uint8).rearrange("p a b c -> p (a b c)"))

    nc.compile()
    return nc


def _host_consts():
    g1 = np.zeros((128, G), dtype=np.float32)
    g2 = np.zeros((128, G), dtype=np.float32)
    for c in range(128):
        g1[c, c // CPG] = 1.0 / CPG
        g2[c, G // 2 + c // CPG] = 1.0 / CPG
    gsel = np.concatenate([g1, g2], axis=1)          # [128, 2G]
    bsel = np.zeros((G, C), dtype=np.float32)
    for c in range(C):
        bsel[c // CPG, c] = 1.0
    return gsel, bsel


def kernel(x, gn_w, gn_b, qkv_w, qkv_b, proj_w, proj_b):
    global LAST_RESULTS
    if "nc" not in _CACHE:
        _CACHE["nc"] = _build_program()
    nc = _CACHE["nc"]

    x = np.ascontiguousarray(np.asarray(x, dtype=np.float32))
    xr = x.reshape(B, C, T)
    gsel, bsel = _host_consts()
    # fold the attention SCALE into the q/k weight columns and biases
    wT = np.ascontiguousarray(np.asarray(qkv_w, np.float32).T)
    wT[:, 0:2 * C] *= SCALE
    qkvb = np.asarray(qkv_b, np.float32).copy()
    qkvb_qk = (qkvb[0:2 * C] * SCALE).reshape(2 * C, 1).copy()
    shared = {
        "wT": wT,
        "qkvb": qkvb_qk,
        "vbrow": qkvb[2 * C:].reshape(1, C).copy(),
        "pjT": np.ascontiguousarray(np.asarray(proj_w, np.float32).T.astype(np.float16)),
        "pjb": np.asarray(proj_b, np.float32).reshape(C, 1).copy(),
        "gnw": np.asarray(gn_w, np.float32).reshape(C, 1).copy(),
        "gnb": np.asarray(gn_b, np.float32).reshape(C, 1).copy(),
        "gsel": gsel,
        "bsel": bsel,
    }
    in_maps = []
    for c in range(NCORES):
        b, hf = c // 2, c % 2
        m = dict(shared)
        # rotate so this core's query half is always columns 0..2047
        xrot = np.roll(xr[b], -hf * THALF, axis=1)
        m["xb"] = np.ascontiguousarray(xrot.astype(np.float16))
        m["xqf"] = np.ascontiguousarray(xrot[:, :THALF])
        in_maps.append(m)

    res = bass_utils.run_bass_kernel_spmd(nc, in_maps, core_ids=list(range(NCORES)))
    LAST_RESULTS = res

    out = np.empty((B, C, T), dtype=np.float32)
    for c in range(NCORES):
        b, hf = c // 2, c % 2
        out[b][:, hf * THALF:(hf + 1) * THALF] = res.results[c]["out"]
    return out.reshape(B, C, HH, WW)


# revision 6
# speedup vs baseline: 1.1696x; 1.0064x over previous
"""AttentionBlock (GroupNorm + 1x1 QKV + MHA + proj + residual) on 8 trn2 cores.

Sharding: core c -> (batch b = c//2, t-half = c%2). Each core computes all 4
heads for its 2048 query positions; k/v are computed over the full T=4096 from
the core's batch. No cross-core communication needed. The host ROTATES each
core's [C, T] input so the core's query half is always columns 0..2047.

v2 design (vs f16 baseline, 331.5us):
- AV matmuls use fp8e4 DoubleRow (contraction 256 per MM): vT and the exp'd
  attention weights (pe) are fp8e4; one DoubleRow MM replaces two f16 MMs
  (measured 216ns per MM, LDWEIGHTS 130 cols hidden).
- exp is the throughput wall (33.5M elements/core); it runs on BOTH psum-
  capable engines: ACT (table Exp -> fp8, whole [128,2,512] chunk) and DVE
  (u8-Schraudolph bit-trick: fp8e4 bits = round(11.54*x + B), saturating
  u8 convert gives exact 0 for tiny weights). GPSIMD cannot read PSUM, so it
  only does the SBUF-side reciprocal broadcast.
- softmax shift M0=2.25 keeps fp8 in range (max logit ~7.0 -> byte 111,
  NaN zone starts 120; logits < M0-4.82 round to weight 0, dropped mass
  ~2e-4 of the denominator) and cancels exactly in the normalize ratio.
- SCALE is folded into the q/k weight columns host-side; the GroupNorm v-bias
  is folded into the proj bias ON DEVICE (pjb_eff = pjb + proj_w @ vb), so vt
  production is a pure psum->fp8 copy that ACT can run.

PSUM (8 banks): ps2 [128,2,512] x3 bufs (QK/exp chunks, production pairs,
vt halves, folds, proj) + acc [128,2,512] x1 (AV accumulator; denominator
row on partition 64 via a ones-column in vT).
"""

import math
from collections import deque

import numpy as np
import ml_dtypes

import concourse.bass as bass
import concourse.tile as tile
from concourse import bacc, mybir
from concourse import bass_utils

F32 = mybir.dt.float32
F16 = mybir.dt.float16
FP8 = mybir.dt.float8e4
U8 = mybir.dt.uint8

B, C, HH, WW = 4, 256, 64, 64
T = HH * WW            # 4096
NH = 4                 # heads per batch
CH = C // NH           # 64 channels per head
G = 32                 # groupnorm groups
CPG = C // G           # 8 channels per group
EPS = 1e-5
NCORES = 8
THALF = T // 2         # 2048  t-columns per core
SCALE = 1.0 / math.sqrt(math.sqrt(CH))
NSC = T // 128         # 32 s-chunks per unit
NPAIR = NSC // 2       # 16 DoubleRow s-pairs
VSPAN = 80             # vt8 per-head column span (16B-aligned pair stride)

M0 = 2.25                                  # softmax shift (cancels in ratio)
EXP_A8 = 8.0 * math.log2(math.e)           # fp8e4m3 Schraudolph multiplier
EXP_B8 = 8.0 * (7.0 - 0.0430) - EXP_A8 * M0

LP = 5                 # AV emission lags this many s-pairs behind QK

_CACHE = {}
LAST_RESULTS = None


def _make_pattern(n_a, n_d):
    """32-slot exp-engine assignment ('A' = ACT, 'D' = DVE); starts 'AA' so
    DVE can drain the previous unit's normalize before its first chunk."""
    assert n_a + n_d == NSC
    out = ["A", "A"]
    rem = {"A": n_a - 2, "D": n_d}
    acc = {k: 0.0 for k in rem}
    tot = sum(rem.values())
    for _ in range(tot):
        counts = {k: sum(1 for c in out[2:] if c == k) for k in rem}
        for k in rem:
            acc[k] += rem[k] / tot
        avail = [k for k in rem if counts[k] < rem[k]]
        pick = max(avail, key=lambda k: acc[k])
        out.append(pick)
        acc[pick] -= 1.0
    assert len(out) == NSC
    return out


UNIT_PATTERNS = []
for u in range(8):
    if u <= 2:
        UNIT_PATTERNS.append(_make_pattern(16, 16))
    elif u == 3:
        UNIT_PATTERNS.append(_make_pattern(17, 15))
    else:
        UNIT_PATTERNS.append(_make_pattern(18, 14))


def _build_program():
    nc = bacc.Bacc("TRN2", target_bir_lowering=False, debug=False)

    d_xb = nc.dram_tensor("xb", [C, T], F16, kind="ExternalInput").ap()
    d_xqf = nc.dram_tensor("xqf", [C, THALF], F32, kind="ExternalInput").ap()
    d_wT = nc.dram_tensor("wT", [C, 3 * C], F32, kind="ExternalInput").ap()
    d_qkvb = nc.dram_tensor("qkvb", [2 * C, 1], F32, kind="ExternalInput").ap()
    d_vbrow = nc.dram_tensor("vbrow", [1, C], F32, kind="ExternalInput").ap()
    d_pjT = nc.dram_tensor("pjT", [C, C], F16, kind="ExternalInput").ap()
    d_pjb = nc.dram_tensor("pjb", [C, 1], F32, kind="ExternalInput").ap()
    d_gnw = nc.dram_tensor("gnw", [C, 1], F32, kind="ExternalInput").ap()
    d_gnb = nc.dram_tensor("gnb", [C, 1], F32, kind="ExternalInput").ap()
    d_gsel = nc.dram_tensor("gsel", [128, 2 * G], F32, kind="ExternalInput").ap()
    d_bsel = nc.dram_tensor("bsel", [G, C], F32, kind="ExternalInput").ap()
    d_scr = nc.dram_tensor("scr", [1, C], F16, kind="Internal").ap()
    d_out = nc.dram_tensor("out", [C, THALF], F32, kind="ExternalOutput").ap()

    with tile.TileContext(nc) as tc:
        with tc.tile_pool(name="data", bufs=1) as data, \
             tc.tile_pool(name="small", bufs=1) as small, \
             tc.tile_pool(name="work", bufs=3) as work:

            # ---- persistent SBUF tensors ----
            x = [data.tile([128, T], F16, tag=f"x{i}", name=f"x{i}") for i in range(2)]
            xqf = [data.tile([128, THALF], F32, tag=f"xqf{i}", name=f"xqf{i}") for i in range(2)]
            wt = [data.tile([128, 3 * C], F32, tag=f"wt{i}", name=f"wt{i}") for i in range(2)]
            wtf = [data.tile([128, 3 * C], F16, tag=f"wtf{i}", name=f"wtf{i}") for i in range(2)]
            pjt = [data.tile([128, C], F16, tag=f"pjt{i}", name=f"pjt{i}") for i in range(2)]
            q_sb = [data.tile([128, THALF], F16, tag=f"q{i}", name=f"q{i}") for i in range(2)]
            k_sb = [data.tile([128, T], F16, tag=f"k{i}", name=f"k{i}") for i in range(2)]
            # vT fp8: per s-chunk, per head: 64 v-columns + 1 ones-column
            vt8 = data.tile([128, NSC, NH, VSPAN], FP8, tag="vt8", name="vt8")
            a_sb = [data.tile([128, THALF], F16, tag=f"a{i}", name=f"a{i}") for i in range(2)]

            gnw = [small.tile([128, 1], F32, tag=f"gnw{i}", name=f"gnw{i}") for i in range(2)]
            gnb = [small.tile([128, 1], F32, tag=f"gnb{i}", name=f"gnb{i}") for i in range(2)]
            pjb = [small.tile([128, 1], F32, tag=f"pjb{i}", name=f"pjb{i}") for i in range(2)]
            qkb = [small.tile([128, 1], F32, tag=f"qkb{o}", name=f"qkb{o}") for o in range(4)]
            gsel = small.tile([128, 2 * G], F32, tag="gsel", name="gsel")
            bsel = small.tile([G, C], F32, tag="bsel", name="bsel")
            vbrow = small.tile([1, C], F32, tag="vbrow", name="vbrow")
            vbT = [small.tile([128, 1], F16, tag=f"vbT{i}", name=f"vbT{i}") for i in range(2)]
            nbias = small.tile([128, 1], F32, tag="nbias", name="nbias")
            nc.vector.memset(nbias[:], -M0)

            # warm the ACT Exp table once
            warm = small.tile([1, 1], F32, tag="warm", name="warm")
            nc.vector.memset(warm[:], 1.0)
            nc.scalar.activation(warm[:], warm[:], mybir.ActivationFunctionType.Exp)

            # ---- DMA: xb first (startup-critical), weights, then the rest.
            nc.sync.dma_start(gsel[:], d_gsel[:, :])
            nc.scalar.dma_start(bsel[:], d_bsel[:, :])
            for ch8 in range(8):
                t8 = slice(512 * ch8, 512 * (ch8 + 1))
                for i in range(2):
                    cs = slice(128 * i, 128 * (i + 1))
                    eng = nc.sync if (ch8 + i) % 2 == 0 else nc.scalar
                    eng.dma_start(x[i][:, t8], d_xb[cs, t8])
            for i in range(2):
                cs = slice(128 * i, 128 * (i + 1))
                (nc.sync if i == 0 else nc.scalar).dma_start(wt[i][:], d_wT[cs, :])
            for i in range(2):
                cs = slice(128 * i, 128 * (i + 1))
                nc.sync.dma_start(gnw[i][:], d_gnw[cs, :])
                nc.sync.dma_start(gnb[i][:], d_gnb[cs, :])
            for o in range(4):
                ob = slice(128 * o, 128 * (o + 1))
                nc.sync.dma_start(qkb[o][:], d_qkvb[ob, :])
            nc.sync.dma_start(vbrow[:], d_vbrow[:, :])
            for i in range(2):
                cs = slice(128 * i, 128 * (i + 1))
                nc.sync.dma_start(pjb[i][:], d_pjb[cs, :])
                nc.sync.dma_start(pjt[i][:], d_pjT[cs, :])
            for i in range(2):
                cs = slice(128 * i, 128 * (i + 1))
                nc.sync.dma_start(xqf[i][:], d_xqf[cs, :])  # residual-only, late

            with tc.tile_pool(name="ps2", bufs=3, space="PSUM") as ps2, \
                 tc.tile_pool(name="psA", bufs=1, space="PSUM") as psA:

                def ps2_tile():
                    return ps2.tile([128, 2, 512], F32, tag="p2", name="p2")

                # PE p-state warmup through the x-DMA wait
                wux = small.tile([128, 512], F16, tag="wux", name="wux")
                nc.vector.memset(wux[:], 0.25)
                wu_ps = ps2_tile()
                for r in range(48):
                    nc.tensor.matmul(wu_ps[0:64, 0, :], wux[:, 0:64], wux[:],
                                     start=True, stop=True)

                # ============ GroupNorm stats -> per-channel A, B ==========
                A = [small.tile([128, 1], F32, tag=f"A{i}", name=f"A{i}") for i in range(2)]
                Bs = [small.tile([128, 1], F32, tag=f"B{i}", name=f"B{i}") for i in range(2)]
                pcs = [small.tile([128, 2], F32, tag=f"pcs{i}", name=f"pcs{i}") for i in range(2)]
                for i in range(2):
                    stats = work.tile([128, 8, 6], F32, tag="bnstats", name="bnstats")
                    for j in range(8):
                        nc.vector.bn_stats(stats[:, j, :], x[i][:, 512 * j:512 * (j + 1)])
                    mv = work.tile([128, 2], F32, tag="bnmv", name="bnmv")
                    nc.vector.bn_aggr(mv[:], stats[:])
                    nc.vector.tensor_copy(pcs[i][:, 0:1], mv[:, 0:1])
                    nc.vector.tensor_tensor(pcs[i][:, 1:2], mv[:, 0:1], mv[:, 0:1],
                                            mybir.AluOpType.mult)
                    nc.vector.tensor_tensor(pcs[i][:, 1:2], pcs[i][:, 1:2], mv[:, 1:2],
                                            mybir.AluOpType.add)

                grp_full = ps2_tile()
                grp_ps = grp_full[0:G, 0, 0:2]
                nc.tensor.matmul(grp_ps, gsel[:, 0:G], pcs[0][:], start=True, stop=False)
                nc.tensor.matmul(grp_ps, gsel[:, G:2 * G], pcs[1][:], start=False, stop=True)

                grp_sb = small.tile([G, 2], F32, tag="grp_sb", name="grp_sb")
                nc.vector.tensor_copy(grp_sb[:], grp_ps)
                grp2 = small.tile([G, 2], F32, tag="grp2", name="grp2")
                var = small.tile([G, 1], F32, tag="var", name="var")
                nc.vector.tensor_copy(grp2[:, 0:1], grp_sb[:, 0:1])
                nc.vector.tensor_tensor(var[:], grp_sb[:, 0:1], grp_sb[:, 0:1],
                                        mybir.AluOpType.mult)
                nc.vector.tensor_tensor(var[:], grp_sb[:, 1:2], var[:],
                                        mybir.AluOpType.subtract)
                nc.vector.tensor_scalar(var[:], var[:], EPS, None,
                                        mybir.AluOpType.add)
                # rstd = rsqrt(var+eps) on DVE: magic seed + Newton iteration
                yr = small.tile([G, 1], F32, tag="yr", name="yr")
                t1 = small.tile([G, 1], mybir.dt.int32, tag="t1", name="t1")
                nc.vector.tensor_scalar(t1[:], var[:].bitcast(mybir.dt.int32),
                                        1, None,
                                        mybir.AluOpType.logical_shift_right)
                nc.vector.tensor_scalar(t1[:], t1[:], 0x5f3759df, -1,
                                        mybir.AluOpType.subtract,
                                        mybir.AluOpType.mult)
                nc.vector.tensor_copy(yr[:].bitcast(mybir.dt.int32), t1[:])
                ytmp = small.tile([G, 1], F32, tag="ytmp", name="ytmp")
                for _ in range(1):
                    nc.vector.tensor_tensor(ytmp[:], yr[:], yr[:],
                                            mybir.AluOpType.mult)
                    nc.vector.tensor_tensor(ytmp[:], ytmp[:], var[:],
                                            mybir.AluOpType.mult)
                    nc.vector.tensor_scalar(ytmp[:], ytmp[:], -0.5, 1.5,
                                            mybir.AluOpType.mult,
                                            mybir.AluOpType.add)
                    nc.vector.tensor_tensor(yr[:], yr[:], ytmp[:],
                                            mybir.AluOpType.mult)
                nc.vector.tensor_copy(grp2[:, 1:2], yr[:])

                # broadcast to channels; A = rstd*gn_w, B = gn_b - mean*A
                for i in range(2):
                    mb_full = ps2_tile()
                    mb_ps = mb_full[:, 0, 0:2]
                    nc.tensor.matmul(mb_ps, bsel[:, 128 * i:128 * (i + 1)], grp2[:],
                                     start=True, stop=True)
                    nc.vector.tensor_tensor(A[i][:], mb_ps[:, 1:2], gnw[i][:],
                                            mybir.AluOpType.mult)
                    nc.vector.tensor_tensor(Bs[i][:], mb_ps[:, 0:1], A[i][:],
                                            mybir.AluOpType.mult)
                    nc.vector.tensor_tensor(Bs[i][:], gnb[i][:], Bs[i][:],
                                            mybir.AluOpType.subtract)
                    nc.vector.tensor_scalar_mul(wtf[i][:], wt[i][:], A[i][:])

                # folded q/k biases (host pre-scaled by SCALE):
                # block o: 0 = q pair0, 1 = q pair1, 2 = k pair0, 3 = k pair1
                def wofs_of(o):
                    return (0 if o < 2 else C) + 128 * (o % 2)

                def fold_qkb(o):
                    b_full = ps2_tile()
                    b_ps = b_full[:, 0, 0:1]
                    wo = wofs_of(o)
                    nc.tensor.matmul(b_ps, wt[0][:, wo:wo + 128], Bs[0][:],
                                     start=True, stop=False)
                    nc.tensor.matmul(b_ps, wt[1][:, wo:wo + 128], Bs[1][:],
                                     start=False, stop=True)
                    nc.vector.tensor_tensor(qkb[o][:], qkb[o][:], b_ps,
                                            mybir.AluOpType.add)

                def fold_vb():
                    # vb = W_v @ B + b_v  (the full per-channel v bias), sent
                    # through DRAM to transpose [1,C] -> 2x[128,1]
                    vb_full = ps2_tile()
                    vb_ps = vb_full[0:1, 0, 0:C]
                    nc.tensor.matmul(vb_ps, Bs[0][:], wt[0][:, 2 * C:3 * C],
                                     start=True, stop=False)
                    nc.tensor.matmul(vb_ps, Bs[1][:], wt[1][:, 2 * C:3 * C],
                                     start=False, stop=True)
                    vb16 = small.tile([1, C], F16, tag="vb16", name="vb16")
                    nc.vector.scalar_tensor_tensor(vb16[:], vb_ps, 1.0, vbrow[:],
                                                   mybir.AluOpType.mult,
                                                   mybir.AluOpType.add)
                    nc.sync.dma_start(d_scr[0:1, :], vb16[:])
                    for i in range(2):
                        nc.sync.dma_start(vbT[i][:], d_scr[0:1, 128 * i:128 * (i + 1)])

                def fold_pjb():
                    # pjb_eff = pjb + proj_w @ vb
                    pe_full = ps2_tile()
                    for o in range(2):
                        pb_ps = pe_full[:, o, 0:1]
                        nc.tensor.matmul(pb_ps, pjt[0][:, 128 * o:128 * (o + 1)],
                                         vbT[0][:], start=True, stop=False)
                        nc.tensor.matmul(pb_ps, pjt[1][:, 128 * o:128 * (o + 1)],
                                         vbT[1][:], start=False, stop=True)
                    for o in range(2):
                        nc.vector.tensor_tensor(pjb[o][:], pjb[o][:],
                                                pe_full[:, o, 0:1],
                                                mybir.AluOpType.add)

                nc.gpsimd.memset(vt8[:, :, :, CH:CH + 1], 1.0)  # denominator col

                # ================= production helpers =====================
                def qk_pair(dst, blk, bias, o, t):
                    # two 512-col t-tiles of q or k output block blk -> dst[o]
                    ts2 = slice(512 * t, 512 * (t + 2))
                    wo = wofs_of(blk)
                    ps = ps2_tile()
                    for u in range(2):
                        ts = slice(512 * (t + u), 512 * (t + u + 1))
                        nc.tensor.matmul(ps[:, u, :], wtf[0][:, wo:wo + 128],
                                         x[0][:, ts], start=True, stop=False)
                        nc.tensor.matmul(ps[:, u, :], wtf[1][:, wo:wo + 128],
                                         x[1][:, ts], start=False, stop=True)
                    dv = dst[o][:, ts2].rearrange("p (u f) -> p u f", u=2)
                    nc.scalar.activation(dv, ps[:],
                                         mybir.ActivationFunctionType.Identity,
                                         bias=bias[:], scale=1.0)

                def vt_tile(v):
                    # s-pair v: vT fp8 for s-chunks 2v, 2v+1 (all 4 heads);
                    # pure copy (v bias folded into pjb_eff)
                    ps = ps2_tile()[:, 0, :].rearrange("p (u f) -> p u f", u=2)
                    for u in range(2):
                        ss = slice(128 * (2 * v + u), 128 * (2 * v + u + 1))
                        nc.tensor.matmul(ps[:, u, :], x[0][:, ss],
                                         wtf[0][:, 2 * C:3 * C],
                                         start=True, stop=False)
                        nc.tensor.matmul(ps[:, u, :], x[1][:, ss],
                                         wtf[1][:, 2 * C:3 * C],
                                         start=False, stop=True)
                    nc.scalar.activation(
                        vt8[:, 2 * v:2 * v + 2, :, 0:CH],
                        ps.rearrange("p u (h c) -> p u h c", h=NH),
                        mybir.ActivationFunctionType.Copy)

                # deferred production, keyed by global group index
                prod_at = {}

                def at(g, fn):
                    prod_at.setdefault(g, []).append(fn)

                # minimal pre-attention production
                fold_qkb(0)
                qk_pair(q_sb, 0, qkb[0], 0, 0)
                fold_qkb(2)
                qk_pair(k_sb, 2, qkb[2], 0, 0)
                fold_vb()
                vt_tile(0)

                # unit 0: vt pairs 1..15 + k0 tiles 1..3
                for v in range(1, 16):
                    at(2 * v - 2, lambda v=v: vt_tile(v))
                at(4, lambda: qk_pair(k_sb, 2, qkb[2], 0, 2))
                at(10, lambda: qk_pair(k_sb, 2, qkb[2], 0, 4))
                at(18, lambda: qk_pair(k_sb, 2, qkb[2], 0, 6))
                at(26, lambda: fold_pjb())
                # units 1-3: q0 second half, k1, q1
                at(34, lambda: qk_pair(q_sb, 0, qkb[0], 0, 2))
                at(40, lambda: (fold_qkb(3), fold_qkb(1)))
                at(44, lambda: qk_pair(k_sb, 3, qkb[3], 1, 0))
                at(52, lambda: qk_pair(k_sb, 3, qkb[3], 1, 2))
                at(66, lambda: qk_pair(k_sb, 3, qkb[3], 1, 4))
                at(74, lambda: qk_pair(k_sb, 3, qkb[3], 1, 6))
                at(80, lambda: qk_pair(q_sb, 1, qkb[1], 1, 0))
                at(100, lambda: qk_pair(q_sb, 1, qkb[1], 1, 2))

                # ================= attention + proj =======================
                with tc.tile_pool(name="pexp", bufs=LP + 3) as pexp, \
                     tc.tile_pool(name="nrm", bufs=2) as nrm, \
                     tc.tile_pool(name="outp", bufs=2) as outp:

                    def emit_av(acc, p, g, pe):
                        for j in range(2):
                            h = 2 * p + j
                            nc.tensor.matmul(
                                acc[0:CH + 1, j, :],
                                vt8[:, 2 * g:2 * g + 2, h, 0:CH + 1],
                                pe[:, :, j, :],
                                start=(g == 0), stop=(g == NPAIR - 1),
                                perf_mode=mybir.MatmulPerfMode.DoubleRow)

                    def normalize(p, tb, acc):
                        tbs = slice(512 * tb, 512 * (tb + 1))
                        # custom DVE ops don't partition-shift: copy the
                        # denominator row to partition 0 first, then recip
                        den = nrm.tile([1, 2, 512], F32, tag="den", name="den")
                        nc.vector.tensor_copy(den[:], acc[CH:CH + 1, :, :])
                        rcp = nrm.tile([1, 2, 512], F32, tag="rcp", name="rcp")
                        nc.vector.reciprocal_approx_fast(rcp[:], den[:])
                        bc = nrm.tile([CH, 2, 512], F32, tag="bc", name="bc")
                        nc.gpsimd.partition_broadcast(bc[:], rcp[:])
                        for j in range(2):
                            hp = slice(64 * j, 64 * j + 64)
                            nc.vector.tensor_tensor(a_sb[p][hp, tbs],
                                                    acc[0:CH, j, :],
                                                    bc[:, j, :],
                                                    mybir.AluOpType.mult)

                    def proj_tb(tb):
                        tbs = slice(512 * tb, 512 * (tb + 1))
                        pr = ps2_tile()
                        for o in range(2):
                            nc.tensor.matmul(pr[:, o, :], pjt[0][:, 128 * o:128 * (o + 1)],
                                             a_sb[0][:, tbs], start=True, stop=False)
                            nc.tensor.matmul(pr[:, o, :], pjt[1][:, 128 * o:128 * (o + 1)],
                                             a_sb[1][:, tbs], start=False, stop=True)
                        for o in range(2):
                            res = outp.tile([128, 512], F32, tag="res", name="res")
                            nc.vector.scalar_tensor_tensor(res[:], pr[:, o, :],
                                                           pjb[o][:],
                                                           xqf[o][:, tbs],
                                                           mybir.AluOpType.add,
                                                           mybir.AluOpType.add)
                            nc.sync.dma_start(d_out[128 * o:128 * (o + 1), tbs], res[:])

                    pend = deque()   # (acc, p, g, pe_tile, post)
                    units = [(p, tb) for p in range(2) for tb in range(THALF // 512)]
                    gg = 0
                    for ui, (p, tb) in enumerate(units):
                        pattern = UNIT_PATTERNS[ui]
                        tbs = slice(512 * tb, 512 * (tb + 1))
                        acc = psA.tile([128, 2, 512], F32, tag="acc", name="acc")
                        pe_cur = None
                        for sc in range(NSC):
                            par = sc % 2
                            g = sc // 2
                            if par == 0:
                                pe_cur = pexp.tile([128, 2, 2, 512], FP8,
                                                   tag="pe", name="pe")
                            scs = slice(128 * sc, 128 * (sc + 1))
                            qt = ps2_tile()
                            for j in range(2):
                                hp = slice(64 * j, 64 * j + 64)
                                nc.tensor.matmul(qt[:, j, :], k_sb[p][hp, scs],
                                                 q_sb[p][hp, tbs],
                                                 start=True, stop=True)
                            if pattern[sc] == "A":
                                nc.scalar.activation(
                                    pe_cur[:, par, :, :], qt[:],
                                    mybir.ActivationFunctionType.Exp,
                                    bias=nbias[:], scale=1.0)
                            else:
                                nc.vector.tensor_scalar(
                                    pe_cur[:, par, :, :].bitcast(U8), qt[:],
                                    EXP_A8, EXP_B8,
                                    mybir.AluOpType.mult, mybir.AluOpType.add)
                            for fn in prod_at.pop(gg, []):
                                fn()
                            if par == 1:
                                post = None
                                if g == NPAIR - 1:
                                    def post(p=p, tb=tb, acc=acc):
                                        normalize(p, tb, acc)
                                        if p == 1:
                                            proj_tb(tb)
                                pend.append((acc, p, g, pe_cur, post))
                                npop = 2 if g >= NPAIR - LP else (
                                    1 if len(pend) > LP else 0)
                                for _ in range(npop):
                                    if not pend:
                                        break
                                    ent = pend.popleft()
                                    emit_av(*ent[:4])
                                    if ent[4] is not None:
                                        ent[4]()
                            gg += 1
                    while pend:
                        ent = pend.popleft()
                        emit_av(*ent[:4])
                        if ent[4] is not None:
                            ent[4]()

    nc.compile()
    return nc


def _host_consts():
    g1 = np.zeros((128, G), dtype=np.float32)
    g2 = np.zeros((128, G), dtype=np.float32)
    for c in range(128):
        g1[c, c // CPG] = 1.0 / CPG
        g2[c, G // 2 + c // CPG] = 1.0 / CPG
    gsel = np.concatenate([g1, g2], axis=1)          # [128, 2G]
    bsel = np.zeros((G, C), dtype=np.float32)
    for c in range(C):
        bsel[c // CPG, c] = 1.0
    return gsel, bsel


def kernel(x, gn_w, gn_b, qkv_w, qkv_b, proj_w, proj_b):
    global LAST_RESULTS
    if "nc" not in _CACHE:
        _CACHE["nc"] = _build_program()
    nc = _CACHE["nc"]

    x = np.ascontiguousarray(np.asarray(x, dtype=np.float32))
    xr = x.reshape(B, C, T)
    gsel, bsel = _host_consts()
    # fold the attention SCALE into the q/k weight columns and biases
    wT = np.ascontiguousarray(np.asarray(qkv_w, np.float32).T)
    wT[:, 0:2 * C] *= SCALE
    qkvb = np.asarray(qkv_b, np.float32).copy()
    qkvb_qk = (qkvb[0:2 * C] * SCALE).reshape(2 * C, 1).copy()
    shared = {
        "wT": wT,
        "qkvb": qkvb_qk,
        "vbrow": qkvb[2 * C:].reshape(1, C).copy(),
        "pjT": np.ascontiguousarray(np.asarray(proj_w, np.float32).T.astype(np.float16)),
        "pjb": np.asarray(proj_b, np.float32).reshape(C, 1).copy(),
        "gnw": np.asarray(gn_w, np.float32).reshape(C, 1).copy(),
        "gnb": np.asarray(gn_b, np.float32).reshape(C, 1).copy(),
        "gsel": gsel,
        "bsel": bsel,
    }
    in_maps = []
    for c in range(NCORES):
        b, hf = c // 2, c % 2
        m = dict(shared)
        # rotate so this core's query half is always columns 0..2047
        xrot = np.roll(xr[b], -hf * THALF, axis=1)
        m["xb"] = np.ascontiguousarray(xrot.astype(np.float16))
        m["xqf"] = np.ascontiguousarray(xrot[:, :THALF])
        in_maps.append(m)

    res = bass_utils.run_bass_kernel_spmd(nc, in_maps, core_ids=list(range(NCORES)))
    LAST_RESULTS = res

    out = np.empty((B, C, T), dtype=np.float32)
    for c in range(NCORES):
        b, hf = c // 2, c % 2
        out[b][:, hf * THALF:(hf + 1) * THALF] = res.results[c]["out"]
    return out.reshape(B, C, HH, WW)
